# revision 7
# baseline (speedup 1.0000x reference)
"""Causal self-attention (B=4, T=2048, C=1024, H=16, D=64) on 8 trn2 NeuronCores.

Sharding: Megatron tensor-parallel over heads. Each core owns 2 heads:
  - Wq/Wk/Wv column-sharded -> per-core [1024, 128] slices
  - attention computed fully on-core for its 2 heads x 4 batches
  - Wo row-sharded -> per-core partial output [1024, 8192] (transposed layout)
  - host sums the 8 partials, adds bo, transposes back.

Device kernel layout notes:
  - All matmuls use float32r (FP22 multiply, fp32 accumulate): full PE rate at
    moving-dim >= 256, ~1e-4 relative error.
  - x is passed transposed (xT [1024, 8192]) so the contraction dim (embed) is
    on partitions for the QKV projections.
  - Q,K are produced transposed ([dims, tokens]); scores are computed
    transposed (scoresT [keys, queries]) so softmax denominators come from a
    ones-row augmentation of V in the PV matmul, and no T x T transpose is
    ever needed.
  - Causal mask: strict-lower-triangle -1e9 add on the 128x128 diagonal
    blocks only; sub-diagonal columns are skipped in the PV accumulation.
"""

import os
import sys

import numpy as np

for _p in ("/opt/trn_rl_repo",):
    if _p not in sys.path and os.path.isdir(_p):
        sys.path.insert(0, _p)

import concourse.bass as bass  # noqa: E402
import concourse.mybir as mybir  # noqa: E402
from concourse import bacc  # noqa: E402
from concourse.masks import make_identity  # noqa: E402
from concourse.tile import TileContext  # noqa: E402
from concourse.bass_utils import run_bass_kernel_spmd  # noqa: E402

B, T, C = 4, 2048, 1024
H, D = 16, 64
NCORES = 8
HPC = H // NCORES          # heads per core = 2
LC = HPC * D               # local channels per core = 128
BT = B * T                 # 8192 tokens
STRIP = 512                # query strip width (= one PSUM bank of fp32)
KT = 128                   # key tile (partition dim)
GROUP = 3                  # key tiles per exp batch (3 PSUM banks)

f32 = mybir.dt.float32
f32r = mybir.dt.float32r

_COMPILED = {}
_LAST_RESULTS = None


def _build(repeat=1):
    nc = bacc.Bacc(None, target_bir_lowering=False)

    xT = nc.dram_tensor("xT", [C, BT], f32r, kind="ExternalInput")
    wq = nc.dram_tensor("wq", [C, LC], f32r, kind="ExternalInput")
    wk = nc.dram_tensor("wk", [C, LC], f32r, kind="ExternalInput")
    wv = nc.dram_tensor("wv", [C, LC], f32r, kind="ExternalInput")
    wo = nc.dram_tensor("wo", [LC, C], f32r, kind="ExternalInput")
    bq = nc.dram_tensor("bq", [LC, 1], f32, kind="ExternalInput")
    bk = nc.dram_tensor("bk", [LC, 1], f32, kind="ExternalInput")
    bv = nc.dram_tensor("bv", [LC, 1], f32, kind="ExternalInput")
    out = nc.dram_tensor("out_pT", [C, BT], f32, kind="ExternalOutput")

    n_strips = T // STRIP            # 4 query strips per batch
    n_kt = T // KT                   # 16 key tiles per batch
    n_ct = C // 128                  # 8 contraction tiles for projections

    with TileContext(nc) as tc:
        with tc.tile_pool(name="const", bufs=1) as constp, \
             tc.tile_pool(name="wpool", bufs=1) as wpool, \
             tc.tile_pool(name="xt", bufs=10) as xtp, \
             tc.tile_pool(name="qk", bufs=2) as qkp, \
             tc.tile_pool(name="va", bufs=2) as vap, \
             tc.tile_pool(name="vt", bufs=2) as vtp, \
             tc.tile_pool(name="ex", bufs=3) as exp_, \
             tc.tile_pool(name="at", bufs=3) as atp, \
             tc.tile_pool(name="nrm", bufs=2) as nrmp, \
             tc.tile_pool(name="ou", bufs=4) as outp, \
             tc.tile_pool(name="scp", bufs=2, space="PSUM") as scp, \
             tc.tile_pool(name="smp", bufs=2, space="PSUM") as smp:

            # ---- constants ----
            tri = constp.tile([128, 128], f32)
            nc.gpsimd.memset(tri[:, :], 0.0)
            # scoresT diag block [key i, query j]: invalid when j < i
            nc.gpsimd.affine_select(
                out=tri[:, :], in_=tri[:, :],
                compare_op=mybir.AluOpType.is_ge, fill=-1e9,
                base=0, pattern=[[1, 128]], channel_multiplier=-1)
            ident = constp.tile([128, 128], f32)
            make_identity(nc, ident[:, :])
            identr = constp.tile([128, 128], f32r)
            nc.vector.tensor_copy(identr[:, :], ident[:, :])

            wq_sb = wpool.tile([128, n_ct * LC], f32r)
            wk_sb = wpool.tile([128, n_ct * LC], f32r)
            wv_sb = wpool.tile([128, n_ct * LC], f32r)
            for k in range(n_ct):
                nc.sync.dma_start(out=wq_sb[:, k * LC:(k + 1) * LC],
                                  in_=wq[k * 128:(k + 1) * 128, :])
                nc.sync.dma_start(out=wk_sb[:, k * LC:(k + 1) * LC],
                                  in_=wk[k * 128:(k + 1) * 128, :])
                nc.sync.dma_start(out=wv_sb[:, k * LC:(k + 1) * LC],
                                  in_=wv[k * 128:(k + 1) * 128, :])
            wo_sb = wpool.tile([128, C], f32r)
            nc.sync.dma_start(out=wo_sb[:, :], in_=wo[:, :])
            bq_sb = wpool.tile([128, 1], f32)
            bk_sb = wpool.tile([128, 1], f32)
            bv_sb = wpool.tile([128, 1], f32)
            nc.sync.dma_start(out=bq_sb[:, :], in_=bq[:, :])
            nc.sync.dma_start(out=bk_sb[:, :], in_=bk[:, :])
            nc.sync.dma_start(out=bv_sb[:, :], in_=bv[:, :])

            for b in [bb % B for bb in range(B * repeat)]:
                t0 = b * T
                # ---- phase A: projections for batch b ----
                qT = qkp.tile([128, T], f32r, tag="qT")
                kTt = qkp.tile([128, T], f32r, tag="kT")
                va = vap.tile([128, n_kt * (D + 1) * HPC], f32r, tag="va")
                # ones columns for the softmax-denominator rows of v_aug
                nc.vector.memset(va[:, :].bitcast(f32), 1.0)

                for st in range(n_strips):
                    q0 = st * STRIP
                    xts = []
                    for k in range(n_ct):
                        xt_t = xtp.tile([128, STRIP], f32r, tag="xt")
                        nc.sync.dma_start(
                            out=xt_t[:, :],
                            in_=xT[k * 128:(k + 1) * 128, t0 + q0:t0 + q0 + STRIP])
                        xts.append(xt_t)
                    # q, k, v projections (sequential; each uses one PSUM slot)
                    for (w_sb, b_sb, kind) in ((wq_sb, bq_sb, "q"),
                                               (wk_sb, bk_sb, "k"),
                                               (wv_sb, bv_sb, "v")):
                        ps = scp.tile([128, GROUP * STRIP], f32, tag="sc")
                        for k in range(n_ct):
                            nc.tensor.matmul(
                                ps[:, 0:STRIP],
                                w_sb[:, k * LC:(k + 1) * LC],
                                xts[k][:, :],
                                start=(k == 0), stop=(k == n_ct - 1))
                        if kind == "q":
                            nc.vector.tensor_scalar_add(
                                qT[:, q0:q0 + STRIP], ps[:, 0:STRIP], b_sb[:, :])
                        elif kind == "k":
                            nc.vector.tensor_scalar_add(
                                kTt[:, q0:q0 + STRIP], ps[:, 0:STRIP], b_sb[:, :])
                        else:
                            vt_t = vtp.tile([128, STRIP], f32r, tag="vt")
                            nc.vector.tensor_scalar_add(
                                vt_t[:, :], ps[:, 0:STRIP], b_sb[:, :])
                            # transpose vT -> v chunks [tokens, dims]
                            for j in range(STRIP // 128):
                                chunk = st * (STRIP // 128) + j
                                cbase = chunk * (D + 1) * HPC
                                for h in range(HPC):
                                    tp = smp.tile([128, STRIP], f32r, tag="pv")
                                    nc.tensor.matmul(
                                        tp[0:128, 0:D],
                                        vt_t[h * D:(h + 1) * D,
                                             j * 128:(j + 1) * 128],
                                        identr[h * D:(h + 1) * D,
                                               h * D:(h + 1) * D],
                                        is_transpose=True, start=True, stop=True)
                                    nc.vector.tensor_copy(
                                        va[:, cbase + h * (D + 1):
                                           cbase + h * (D + 1) + D],
                                        tp[0:128, 0:D])

                # ---- phase B: attention + output projection for batch b ----
                for st in range(n_strips):
                    q0 = st * STRIP
                    at_t = atp.tile([128, STRIP], f32r, tag="at")
                    for h in range(HPC):
                        hb = h * D
                        pv = smp.tile([128, STRIP], f32, tag="pv")
                        smax = (q0 + STRIP) // KT
                        s = 0
                        while s < smax:
                            group = list(range(s, min(s + GROUP, smax)))
                            sc_t = scp.tile([128, GROUP * STRIP], f32, tag="sc")
                            for i, si in enumerate(group):
                                nc.tensor.matmul(
                                    sc_t[:, i * STRIP:(i + 1) * STRIP],
                                    kTt[hb:hb + D, si * KT:(si + 1) * KT],
                                    qT[hb:hb + D, q0:q0 + STRIP],
                                    start=True, stop=True)
                                off = si * KT - q0
                                if off >= 0:
                                    nc.vector.tensor_tensor(
                                        out=sc_t[:, i * STRIP + off:
                                                 i * STRIP + off + 128],
                                        in0=sc_t[:, i * STRIP + off:
                                                 i * STRIP + off + 128],
                                        in1=tri[:, :], op=mybir.AluOpType.add)
                            wdt = len(group) * STRIP
                            ex_t = exp_.tile([128, GROUP * STRIP], f32r, tag="ex")
                            nc.scalar.activation(
                                ex_t[:, 0:wdt], sc_t[:, 0:wdt],
                                mybir.ActivationFunctionType.Exp, scale=0.125)
                            for i, si in enumerate(group):
                                off = max(0, si * KT - q0)
                                cb = si * (D + 1) * HPC + h * (D + 1)
                                nc.tensor.matmul(
                                    pv[0:D + 1, off:STRIP],
                                    va[:, cb:cb + D + 1],
                                    ex_t[:, i * STRIP + off:(i + 1) * STRIP],
                                    start=(si == 0), stop=(si == smax - 1))
                            s += GROUP
                        # normalize: out[d, q] / denom[q]
                        rc = nrmp.tile([1, STRIP], f32, tag="rc")
                        nc.vector.reciprocal(rc[0:1, :], pv[D:D + 1, :])
                        bc = nrmp.tile([64, STRIP], f32, tag="bc")
                        nc.gpsimd.partition_broadcast(bc[0:64, :], rc[0:1, :])
                        nc.vector.tensor_tensor(
                            out=at_t[hb:hb + D, :], in0=pv[0:D, :],
                            in1=bc[:, :], op=mybir.AluOpType.mult)
                    # output projection for this strip
                    for od in range(C // 128):
                        pj = smp.tile([128, STRIP], f32, tag="pv")
                        nc.tensor.matmul(
                            pj[:, :],
                            wo_sb[:, od * 128:(od + 1) * 128],
                            at_t[:, :], start=True, stop=True)
                        ot = outp.tile([128, STRIP], f32, tag="ou")
                        if od % 2 == 0:
                            nc.vector.tensor_copy(ot[:, :], pj[:, :])
                        else:
                            nc.scalar.copy(ot[:, :], pj[:, :])
                        nc.sync.dma_start(
                            out=out[od * 128:(od + 1) * 128,
                                    t0 + q0:t0 + q0 + STRIP],
                            in_=ot[:, :])

    nc.compile()
    return nc


def _get_compiled(repeat=1):
    if repeat not in _COMPILED:
        _COMPILED[repeat] = _build(repeat)
    return _COMPILED[repeat]


def kernel(x, mask, Wq, bq, Wk, bk, Wv, bv, Wo, bo, _repeat=1):
    global _LAST_RESULTS
    x = np.asarray(x, dtype=np.float32)
    Wq = np.asarray(Wq, dtype=np.float32)
    Wk = np.asarray(Wk, dtype=np.float32)
    Wv = np.asarray(Wv, dtype=np.float32)
    Wo = np.asarray(Wo, dtype=np.float32)
    bq = np.asarray(bq, dtype=np.float32)
    bk = np.asarray(bk, dtype=np.float32)
    bv = np.asarray(bv, dtype=np.float32)
    bo = np.asarray(bo, dtype=np.float32)

    nc = _get_compiled(_repeat)
    xT = np.ascontiguousarray(x.reshape(BT, C).T)

    in_maps = []
    for c in range(NCORES):
        lo, hi = c * LC, (c + 1) * LC
        in_maps.append({
            "xT": xT,
            "wq": np.ascontiguousarray(Wq[:, lo:hi]),
            "wk": np.ascontiguousarray(Wk[:, lo:hi]),
            "wv": np.ascontiguousarray(Wv[:, lo:hi]),
            "wo": np.ascontiguousarray(Wo[lo:hi, :]),
            "bq": np.ascontiguousarray(bq[lo:hi].reshape(LC, 1)),
            "bk": np.ascontiguousarray(bk[lo:hi].reshape(LC, 1)),
            "bv": np.ascontiguousarray(bv[lo:hi].reshape(LC, 1)),
        })

    import time as _time
    trace = bool(os.environ.get("BASS_KERNEL_TRACE"))
    t0 = _time.time()
    res = run_bass_kernel_spmd(nc, in_maps, core_ids=list(range(NCORES)),
                               trace=trace)
    kernel.last_exec_wall = _time.time() - t0
    _LAST_RESULTS = res

    total = res.results[0]["out_pT"].astype(np.float64)
    for c in range(1, NCORES):
        total += res.results[c]["out_pT"]
    total += bo.astype(np.float64)[:, None]
    return np.ascontiguousarray(total.T).reshape(B, T, C).astype(np.float32)
